# revision 25
# baseline (speedup 1.0000x reference)
"""Multi-head channel-attention kernel for Trainium2 (8 NeuronCores, SPMD).

Reference computation (per batch b, x = [256, N] with N = 64*64 = 4096):
    qkv   = w_qkv @ x
    q,k,v = per-head [256, N] slices of qkv
    logit = (q*scale) @ k.T          # [256, 256] (contraction over N)
    wts   = softmax(logit, -1)
    out_h = wts @ v
    y     = w_out @ stack_h(out_h) + b_out

Distribution: pure data-parallel — batch 8 across 8 cores, one batch per
core, no collectives.

The kernel exploits that attention is over the *channel* axis (n >> c):

    logit_h = (Wq_h * scale) @ (x @ x.T) @ Wk_h.T
    y       = (sum_h W_h @ softmax_h @ Wv_h) @ x + b  =  Wstar @ x + b

so the only n-wide work is the Gram matrix G = x @ x.T (one pass over x)
and the final Wstar @ x (second pass). Everything else is [256,256]-sized.
Per-batch FLOPs drop from 12.9G (direct) to 1.6G.

Pipeline (all matmuls TensorE, bf16 operands, fp32 PSUM):
    G    = xT.T @ xT                  (xT shipped pre-transposed from host)
    A_h  = G @ Wk_h.T                 (uses G's symmetry: lhsT = G)
    L_h  = (Wq_h*scale) @ A_h         -> PSUM
    E_h  = exp(L_h) on ScalarE straight from PSUM, accum_out = row sums;
           row-normalize with VectorE reciprocal (softmax; logits are O(1)
           for this problem so no max-subtraction is needed)
    M_hT = E_h-contraction with WoT   (computed directly transposed:
           lhsT = Ehat, rhs = WoT — no on-chip transposes anywhere)
    WstarT = sum_h Wv_h-contraction with M_hT
    y    = WstarT.T @ x + b           (bias folded into the PSUM drains,
           which alternate VectorE/ScalarE so they keep up with the PE)

The four [256,256]-per-head stages are software-pipelined across heads
(emission order A0 A1 L0 A2 L1 M0 A3 L2 M1 L3 M2 M3 Wst) so the PE never
waits on the softmax chain of the head in flight.

Weights are repacked/pre-transposed on the host; x is shipped twice (native
and transposed, bf16) so the Gram matmul needs no on-chip transpose. Input
DMAs are packed into 9 descriptors, outputs into 4.
"""

import numpy as np
import ml_dtypes

import concourse.bass as bass
import concourse.mybir as mybir
import concourse.tile as tile
from concourse.bass import ts
from concourse.bass_utils import run_bass_kernel_spmd
from concourse.vector_clock import ScopedClock

B, DIM, H, W = 8, 256, 64, 64
HEADS = 4
N = H * W            # 4096
P = 128
KT = DIM // P        # 2 channel tiles
NT = N // P          # 32 n-tiles of 128
NQ = 4               # xT shipped in 4 slabs of 8 n-tiles
NCH = N // 512       # 8 n-chunks of 512
N_CORES = 8

F32 = mybir.dt.float32
BF16 = mybir.dt.bfloat16
NPBF16 = ml_dtypes.bfloat16


def _split_multi_waits(nc, max_waits=1):
    """The walrus build in this container rejects instructions carrying more
    than one sync-wait. Move excess waits onto same-engine carrier NOPs
    inserted immediately before the instruction (engines are in-order, so
    waiting earlier on the same stream is equivalent)."""
    n_split = 0
    for f in nc.m.functions:
        for bb in f.blocks:
            old = list(bb.instructions)
            new = []
            changed = False
            for inst in old:
                si = inst.sync_info
                waits = list(si.on_wait) if si and si.on_wait else []
                if len(waits) > max_waits:
                    changed = True
                    for w in waits[max_waits:]:
                        n_split += 1
                        new.append(
                            mybir.InstNoOp(
                                name=f"wsplit_{n_split}_{inst.name}",
                                engine=inst.engine,
                                ins=[],
                                outs=[],
                                sync_info=mybir.SyncInfo(on_wait=[w], on_update=[]),
                            )
                        )
                    inst.sync_info = mybir.SyncInfo(
                        on_wait=waits[:max_waits], on_update=si.on_update
                    )
                new.append(inst)
            if changed:
                bb.instructions = new
    return n_split


def _minimal_exit(self, tick_clock, wait_clock):
    """TileContext._drain_and_barrier replacement: one SP drain carrying the
    global-clock waits (split onto NOPs by _split_multi_waits afterwards).

    The stock exit adds two all-engine barriers and ~200 per-semaphore
    clears (~10 us). They are redundant here: the bass preamble range-clears
    the whole kernel semaphore range at startup, and bass's own postamble
    still drains every engine.
    """
    nc = self.nc
    drain = nc.sync.drain()
    wait_clock.add_sem_waits(drain.ins, ScopedClock({None: tick_clock.global_clock}))
    popped = nc._tile_sem_poison_stack.pop()
    assert popped is self._sem_poison


def _hoist_to_main(nc):
    """Move dependency-free startup work (input-DMA triggers, PE warmup
    matmuls and their memset) from the tile block into `main`, before the
    startup all-engine barrier. The engines boot staggered over ~3-7 us;
    hoisted work overlaps that skew, so the input DMAs are in flight and
    the PE clock-gate is released by the time the real kernel body starts.
    Semaphores are cleared by the runtime at load, so pre-barrier sem
    increments are observed correctly by post-barrier waiters."""
    names = set(getattr(nc, "_hoist_names", []))
    if not names:
        return
    f = nc.m.functions[0]
    main = f.blocks[0]
    moved = []
    for b in f.blocks[1:]:
        old = list(b.instructions)
        got = [i for i in old if i.name in names]
        if got:
            b.instructions = [i for i in old if i.name not in names]
            moved.extend(got)
    main_list = list(main.instructions)
    # insert after the const-AP memsets, before the startup barrier drains
    pos = next(
        (
            idx
            for idx, i in enumerate(main_list)
            if i.sync_info
            and any("barrier" in (w.ant_name or "") for w in (i.sync_info.on_wait or []))
        ),
        len(main_list),
    )
    main.instructions = main_list[:pos] + moved + main_list[pos:]


def build_program():
    """Build the single-core Bass program (run SPMD across 8 cores)."""
    nc = bass.Bass()

    x_d = nc.declare_dram_parameter("x", [DIM, N], BF16, isOutput=False)
    # xt: [NQ][128, 8, 256]; slab qi, element (p, a, c) = x.T[qi*1024 + a*128 + p, c]
    xt_d = nc.declare_dram_parameter("xt", [NQ, P, NT // NQ, DIM], BF16, isOutput=False)
    # wpk: [KT][128, 4096] = [wqT | wkT | wv | woT], each [128, HEADS*256]
    wpk_d = nc.declare_dram_parameter("wpk", [KT, P, 4 * HEADS * DIM], BF16, isOutput=False)
    b_d = nc.declare_dram_parameter("b", [DIM, 1], F32, isOutput=False)
    y_d = nc.declare_dram_parameter("y", [DIM, N], F32, isOutput=True)

    OQT, OKT, OV, OOT = 0, HEADS * DIM, 2 * HEADS * DIM, 3 * HEADS * DIM

    prev_exit = tile.TileContext._drain_and_barrier
    tile.TileContext._drain_and_barrier = _minimal_exit
    try:
        _build_body(nc, tc_args=(x_d, xt_d, wpk_d, b_d, y_d, OQT, OKT, OV, OOT))
    finally:
        tile.TileContext._drain_and_barrier = prev_exit

    # NOTE: hoisting startup work before the init barrier was tried and lost
    # time — the runtime preamble (~6.5us) gates all engines anyway, and
    # pre-barrier work just delays the barrier release for everyone.
    _split_multi_waits(nc)
    return nc


def _build_body(nc, tc_args):
    x_d, xt_d, wpk_d, b_d, y_d, OQT, OKT, OV, OOT = tc_args
    with tile.TileContext(nc) as tc:
        with (
            tc.tile_pool(name="wpool", bufs=1) as wpool,
            tc.tile_pool(name="spool", bufs=2) as spool,
            tc.tile_pool(name="ypool", bufs=2) as ypool,
            tc.tile_pool(name="psum", bufs=1, space="PSUM") as psum,
        ):
            hoist = []

            # ---- PE warmup: ~3.5us of dummy matmuls during the input DMAs
            # releases the HAM clock-gate so G runs at 2.4 GHz from its
            # first instruction.
            warm = wpool.tile([P, P], BF16, tag="warm")
            hoist.append(nc.gpsimd.memset(warm[:], 0).ins.name)
            wps = psum.tile([P, P], F32, tag="g0", bufs=1)
            for _ in range(32):
                hoist.append(
                    nc.tensor.matmul(wps[:], warm[:], warm[:], start=True, stop=True).ins.name
                )

            # ---- loads (xT slabs first: G consumes them incrementally;
            # triggers split across the two HWDGE engines (SP + ACT) so the
            # trigger chains run in parallel) ----
            xt_sb = []
            for qi in range(NQ):
                t = wpool.tile([P, NT // NQ, DIM], BF16, tag=f"xt{qi}")
                eng = nc.sync if qi % 2 == 0 else nc.scalar
                hoist.append(eng.dma_start(t[:], xt_d[qi]).ins.name)
                xt_sb.append(t)
            wpk_sb = []
            for k in range(KT):
                t = wpool.tile([P, 4 * HEADS * DIM], BF16, tag=f"wpk{k}")
                eng = nc.sync if k == 0 else nc.scalar
                hoist.append(eng.dma_start(t[:], wpk_d[k]).ins.name)
                wpk_sb.append(t)
            b_sb = []
            for ot in range(KT):
                t = wpool.tile([P, 1], F32, tag=f"b{ot}")
                eng = nc.sync if ot == 0 else nc.scalar
                hoist.append(eng.dma_start(t[:], b_d[ts(ot, P), :]).ins.name)
                b_sb.append(t)
            x_sb = []
            for k in range(KT):
                t = wpool.tile([P, N], BF16, tag=f"x{k}")
                eng = nc.sync if k == 0 else nc.scalar
                hoist.append(eng.dma_start(t[:], x_d[ts(k, P), :]).ins.name)
                x_sb.append(t)
            nc._hoist_names = hoist

            # ---- G = x @ x.T (fp32 PSUM, 32 accumulation steps) ----------
            g_ps = []
            for ct in range(KT):
                gp = psum.tile([P, DIM], F32, tag=f"g{ct}", bufs=1)
                g_ps.append(gp)
            for i in range(NT):
                qi, a = divmod(i, NT // NQ)
                for ct in range(KT):
                    nc.tensor.matmul(
                        g_ps[ct][:],
                        xt_sb[qi][:, a, ts(ct, P)],
                        xt_sb[qi][:, a, :],
                        start=(i == 0),
                        stop=(i == NT - 1),
                    )
            g_sb = []
            for ct in range(KT):
                g = spool.tile([P, DIM], BF16, tag=f"gs{ct}", bufs=1)
                nc.any.tensor_copy(g[:], g_ps[ct][:])
                g_sb.append(g)

            # ---- per-head stages, software-pipelined across heads --------
            # stage A(h): A = G @ Wk_h.T          (PE + drain)
            # stage L(h): L = (Wq_h*scale) @ A    (PE -> PSUM) + softmax
            # stage M(h): M_hT = Ehat . WoT       (PE + drain)
            a_all, es_all, lp_all = {}, {}, {}
            m_sb = {}

            def stage_A(h):
                a_sb = []
                for ct in range(KT):
                    ap = psum.tile([P, DIM], F32, tag="a", bufs=2, name=f"ap{h}_{ct}")
                    for k in range(KT):
                        # A[c', d] = sum_c'' G[c'', c'] wkT[c'', d]  (G symmetric)
                        nc.tensor.matmul(
                            ap[:],
                            g_sb[k][:, ts(ct, P)],
                            wpk_sb[k][:, OKT + h * DIM : OKT + (h + 1) * DIM],
                            start=(k == 0),
                            stop=(k == KT - 1),
                        )
                    at = spool.tile([P, DIM], BF16, tag=f"a{ct}", name=f"at{h}_{ct}")
                    nc.any.tensor_copy(at[:], ap[:])
                    a_sb.append(at)
                a_all[h] = a_sb

            def stage_L(h):
                pl = []
                for ct in range(KT):
                    lp = psum.tile([P, DIM], F32, tag=f"l{ct}", bufs=1, name=f"lp{h}_{ct}")
                    for k in range(KT):
                        # L[c, d] = sum_c' wqT[c', c] A[c', d]
                        nc.tensor.matmul(
                            lp[:],
                            wpk_sb[k][:, OQT + h * DIM + ct * P : OQT + h * DIM + (ct + 1) * P],
                            a_all[h][k][:],
                            start=(k == 0),
                            stop=(k == KT - 1),
                        )
                    pl.append(lp)
                lp_all[h] = pl
                # softmax immediately (ACT/DVE; doesn't occupy the PE)
                es = []
                for ct in range(KT):
                    e = spool.tile([P, DIM], BF16, tag=f"e{ct}", name=f"e{h}_{ct}")
                    s = spool.tile([P, 1], F32, tag=f"s{ct}", name=f"s{h}_{ct}")
                    r = spool.tile([P, 1], F32, tag=f"r{ct}", name=f"r{h}_{ct}")
                    nc.scalar.activation(
                        e[:], pl[ct][:], mybir.ActivationFunctionType.Exp,
                        accum_out=s[:],
                    )
                    nc.vector.reciprocal(r[:], s[:])
                    nc.any.tensor_scalar_mul(e[:], e[:], r[:])
                    es.append(e)
                es_all[h] = es

            def stage_M(h):
                es = es_all[h]
                for dt2 in range(KT):
                    pm = psum.tile([P, DIM], F32, tag="m", bufs=2, name=f"pm{h}_{dt2}")
                    for ct in range(KT):
                        # M_hT[d, o] = sum_c Ehat[c, d] woT[c, o]
                        nc.tensor.matmul(
                            pm[:],
                            es[ct][:, ts(dt2, P)],
                            wpk_sb[ct][:, OOT + h * DIM : OOT + (h + 1) * DIM],
                            start=(ct == 0),
                            stop=(ct == KT - 1),
                        )
                    mt = spool.tile([P, DIM], BF16, tag=f"m{h}_{dt2}", bufs=1,
                                    name=f"mt{h}_{dt2}")
                    m_sb[(h, dt2)] = mt
                    nc.any.tensor_copy(mt[:], pm[:])

            # pipelined emission: PE order A0 A1 L0 A2 L1 M0 A3 L2 M1 L3 M2 M3
            stage_A(0)
            stage_A(1)
            stage_L(0)
            stage_A(2)
            stage_L(1)
            stage_M(0)
            stage_A(3)
            stage_L(2)
            stage_M(1)
            stage_L(3)
            stage_M(2)
            stage_M(3)

            # ---- WstarT[c_in, o] = sum_h sum_d wv[d, c_in] M_hT[d, o] ----
            wst_sb = []
            for ct in range(KT):
                wp = psum.tile([P, DIM], F32, tag="m", bufs=2, name=f"wp{ct}")
                first = True
                for h in range(HEADS):
                    for dt2 in range(KT):
                        nc.tensor.matmul(
                            wp[:],
                            wpk_sb[dt2][:, OV + h * DIM + ct * P : OV + h * DIM + (ct + 1) * P],
                            m_sb[(h, dt2)][:],
                            start=first,
                            stop=(h == HEADS - 1 and dt2 == KT - 1),
                        )
                        first = False
                wt = spool.tile([P, DIM], BF16, tag=f"wst{ct}", bufs=1, name=f"wt{ct}")
                nc.any.tensor_copy(wt[:], wp[:])
                wst_sb.append(wt)

            # ---- y = WstarT.T @ x + b ------------------------------------
            # drains alternate DVE/ACT; 4 output DMAs of [128, 2048] each
            y_sb = {}
            for ot in range(KT):
                for half in range(4):
                    y_sb[(ot, half)] = ypool.tile(
                        [P, 2 * 512], F32, tag=f"y{ot}_{half}", bufs=1,
                        name=f"ysb{ot}_{half}",
                    )
            ycnt = 0
            ytags = ["g0", "g1", "a", "a"]
            for j in range(NCH):
                for ot in range(KT):
                    py = psum.tile([P, 512], F32, tag=ytags[ycnt % 4],
                                   bufs=(1 if ycnt % 4 < 2 else 2),
                                   name=f"py{j}_{ot}")
                    for k in range(KT):
                        nc.tensor.matmul(
                            py[:],
                            wst_sb[k][:, ts(ot, P)],
                            x_sb[k][:, ts(j, 512)],
                            start=(k == 0),
                            stop=(k == KT - 1),
                        )
                    half, jj = divmod(j, 2)
                    dst = y_sb[(ot, half)][:, ts(jj, 512)]
                    if ycnt % 2 == 0:
                        nc.vector.tensor_scalar_add(dst, py[:], b_sb[ot][:])
                    else:
                        nc.scalar.add(dst, py[:], b_sb[ot][:])
                    ycnt += 1
                    if jj == 1:
                        nc.sync.dma_start(
                            y_d[ts(ot, P), half * 1024 : (half + 1) * 1024],
                            y_sb[(ot, half)][:],
                        )


def prep_inputs(x, w_qkv, w_out, b_out):
    """Host-side packing: per-core input dicts (numpy only)."""
    x = np.asarray(x, dtype=np.float32)
    w_qkv = np.asarray(w_qkv, dtype=np.float32)
    w_out = np.asarray(w_out, dtype=np.float32)
    b_out = np.asarray(b_out, dtype=np.float32)

    scale = float(DIM) ** -0.5
    wq = w_qkv[0 * HEADS * DIM : 1 * HEADS * DIM].reshape(HEADS, DIM, DIM)
    wk = w_qkv[1 * HEADS * DIM : 2 * HEADS * DIM].reshape(HEADS, DIM, DIM)
    wv = w_qkv[2 * HEADS * DIM : 3 * HEADS * DIM].reshape(HEADS, DIM, DIM)

    # wqT[c', h*256 + c] = wq[h, c, c'] * scale
    wqT = (np.transpose(wq, (2, 0, 1)) * scale).reshape(DIM, HEADS * DIM)
    # wkT[c', h*256 + d] = wk[h, d, c']
    wkT = np.transpose(wk, (2, 0, 1)).reshape(DIM, HEADS * DIM)
    # wvn[d, h*256 + c_in] = wv[h, d, c_in]  (natural orientation, head-concat)
    wvn = np.transpose(wv, (1, 0, 2)).reshape(DIM, HEADS * DIM)
    # woT[c, h*256 + o] = w_out[o, c*HEADS + h]
    woT = np.ascontiguousarray(
        w_out.reshape(DIM, DIM, HEADS).transpose(1, 2, 0)
    ).reshape(DIM, HEADS * DIM)

    # wpk[k] = [wqT | wkT | wv | woT] rows k*128:(k+1)*128
    wpk = np.concatenate([wqT, wkT, wvn, woT], axis=1).astype(NPBF16)
    wpk = np.ascontiguousarray(wpk.reshape(KT, P, 4 * HEADS * DIM))
    b = b_out.reshape(DIM, 1).astype(np.float32)

    in_maps = []
    for bi in range(B):
        xb = np.ascontiguousarray(x[bi].reshape(DIM, N)).astype(NPBF16)
        # xt[qi, p, a, c] = x.T[qi*1024 + a*128 + p, c]
        xt = np.ascontiguousarray(
            xb.T.reshape(NQ, NT // NQ, P, DIM).transpose(0, 2, 1, 3)
        )
        in_maps.append({"x": xb, "xt": xt, "wpk": wpk, "b": b})
    return in_maps


_NC_CACHE = {}


def get_program():
    if "nc" not in _NC_CACHE:
        _NC_CACHE["nc"] = build_program()
    return _NC_CACHE["nc"]


def kernel(x, w_qkv, w_out, b_out, **_unused):
    nc = get_program()
    in_maps = prep_inputs(x, w_qkv, w_out, b_out)
    res = run_bass_kernel_spmd(nc, in_maps, list(range(N_CORES)))
    y = np.stack([res.results[c]["y"] for c in range(N_CORES)], axis=0)
    return y.reshape(B, DIM, H, W).astype(np.float32)


# revision 31
# speedup vs baseline: 1.1082x; 1.1082x over previous
"""Multi-head channel-attention kernel for Trainium2 (8 NeuronCores, SPMD).

Reference computation (per batch b, x = [256, N] with N = 64*64 = 4096):
    qkv   = w_qkv @ x
    q,k,v = per-head [256, N] slices of qkv
    logit = (q*scale) @ k.T          # [256, 256] (contraction over N)
    wts   = softmax(logit, -1)
    out_h = wts @ v
    y     = w_out @ stack_h(out_h) + b_out

Distribution: pure data-parallel — batch 8 across 8 cores, one batch per
core, no collectives.

The kernel exploits that attention is over the *channel* axis (n >> c):

    logit_h = (Wq_h * scale) @ (x @ x.T) @ Wk_h.T
    y       = (sum_h W_h @ softmax_h @ Wv_h) @ x + b  =  Wstar @ x + b

so the only n-wide work is the Gram matrix G = x @ x.T (one pass over x)
and the final Wstar @ x (second pass). Everything else is [256,256]-sized.
Per-batch FLOPs drop from 12.9G (direct) to 1.6G.

Pipeline (all matmuls TensorE, bf16 operands, fp32 PSUM):
    G    = xT.T @ xT                  (xT shipped pre-transposed from host)
    A_h  = G @ Wk_h.T                 (uses G's symmetry: lhsT = G)
    L_h  = (Wq_h*scale) @ A_h         -> PSUM
    E_h  = exp(L_h) on ScalarE straight from PSUM, accum_out = row sums;
           row-normalize with VectorE reciprocal (softmax; logits are O(1)
           for this problem so no max-subtraction is needed)
    M_hT = E_h-contraction with WoT   (computed directly transposed:
           lhsT = Ehat, rhs = WoT — no on-chip transposes anywhere)
    WstarT = sum_h Wv_h-contraction with M_hT
    y    = WstarT.T @ x + b           (bias folded into the PSUM drains,
           which alternate VectorE/ScalarE so they keep up with the PE)

The four [256,256]-per-head stages are software-pipelined across heads
(emission order A0 A1 L0 A2 L1 M0 A3 L2 M1 L3 M2 M3 Wst) so the PE never
waits on the softmax chain of the head in flight.

Weights are repacked/pre-transposed on the host; x is shipped twice (native
and transposed, bf16) so the Gram matmul needs no on-chip transpose. Input
DMAs are packed into 9 descriptors, outputs into 4.
"""

import numpy as np
import ml_dtypes

import concourse.bass as bass
import concourse.mybir as mybir
import concourse.tile as tile
from concourse.bass import ts
from concourse.bass_utils import run_bass_kernel_spmd
from concourse.vector_clock import ScopedClock

B, DIM, H, W = 8, 256, 64, 64
HEADS = 4
N = H * W            # 4096
P = 128
KT = DIM // P        # 2 channel tiles
NT = N // P          # 32 n-tiles of 128
NQ = 8               # xT shipped in 8 slabs of 4 n-tiles
NCH = N // 512       # 8 n-chunks of 512
N_CORES = 8

F32 = mybir.dt.float32
BF16 = mybir.dt.bfloat16
NPBF16 = ml_dtypes.bfloat16


def _split_multi_waits(nc, max_waits=1):
    """The walrus build in this container rejects instructions carrying more
    than one sync-wait. Move excess waits onto same-engine carrier NOPs
    inserted immediately before the instruction (engines are in-order, so
    waiting earlier on the same stream is equivalent)."""
    n_split = 0
    for f in nc.m.functions:
        for bb in f.blocks:
            old = list(bb.instructions)
            new = []
            changed = False
            for inst in old:
                si = inst.sync_info
                waits = list(si.on_wait) if si and si.on_wait else []
                if len(waits) > max_waits:
                    changed = True
                    for w in waits[max_waits:]:
                        n_split += 1
                        new.append(
                            mybir.InstNoOp(
                                name=f"wsplit_{n_split}_{inst.name}",
                                engine=inst.engine,
                                ins=[],
                                outs=[],
                                sync_info=mybir.SyncInfo(on_wait=[w], on_update=[]),
                            )
                        )
                    inst.sync_info = mybir.SyncInfo(
                        on_wait=waits[:max_waits], on_update=si.on_update
                    )
                new.append(inst)
            if changed:
                bb.instructions = new
    return n_split


def _minimal_exit(self, tick_clock, wait_clock):
    """TileContext._drain_and_barrier replacement: one SP drain carrying the
    global-clock waits (split onto NOPs by _split_multi_waits afterwards).

    The stock exit adds two all-engine barriers and ~200 per-semaphore
    clears (~10 us). They are redundant here: the bass preamble range-clears
    the whole kernel semaphore range at startup, and bass's own postamble
    still drains every engine.
    """
    nc = self.nc
    drain = nc.sync.drain()
    wait_clock.add_sem_waits(drain.ins, ScopedClock({None: tick_clock.global_clock}))
    popped = nc._tile_sem_poison_stack.pop()
    assert popped is self._sem_poison


def _hoist_to_main(nc):
    """Move dependency-free startup work (input-DMA triggers, PE warmup
    matmuls and their memset) from the tile block into `main`, before the
    startup all-engine barrier. The engines boot staggered over ~3-7 us;
    hoisted work overlaps that skew, so the input DMAs are in flight and
    the PE clock-gate is released by the time the real kernel body starts.
    Semaphores are cleared by the runtime at load, so pre-barrier sem
    increments are observed correctly by post-barrier waiters."""
    names = set(getattr(nc, "_hoist_names", []))
    if not names:
        return
    f = nc.m.functions[0]
    main = f.blocks[0]
    moved = []
    for b in f.blocks[1:]:
        old = list(b.instructions)
        got = [i for i in old if i.name in names]
        if got:
            b.instructions = [i for i in old if i.name not in names]
            moved.extend(got)
    main_list = list(main.instructions)
    # insert after the const-AP memsets, before the startup barrier drains
    pos = next(
        (
            idx
            for idx, i in enumerate(main_list)
            if i.sync_info
            and any("barrier" in (w.ant_name or "") for w in (i.sync_info.on_wait or []))
        ),
        len(main_list),
    )
    main.instructions = main_list[:pos] + moved + main_list[pos:]


def build_program():
    """Build the single-core Bass program (run SPMD across 8 cores)."""
    nc = bass.Bass()

    x_d = nc.declare_dram_parameter("x", [DIM, N], BF16, isOutput=False)
    # xt: [NQ][128, 8, 256]; slab qi, element (p, a, c) = x.T[qi*1024 + a*128 + p, c]
    xt_d = nc.declare_dram_parameter("xt", [NQ, P, NT // NQ, DIM], BF16, isOutput=False)
    # wkq: [KT][128, 2048] = [wqT | wkT]; wvo: [KT][128, 2048] = [wv | woT]
    wkq_d = nc.declare_dram_parameter("wkq", [KT, P, 2 * HEADS * DIM], BF16, isOutput=False)
    wvo_d = nc.declare_dram_parameter("wvo", [KT, P, 2 * HEADS * DIM], BF16, isOutput=False)
    b_d = nc.declare_dram_parameter("b", [DIM, 1], F32, isOutput=False)
    y_d = nc.declare_dram_parameter("y", [DIM, N], F32, isOutput=True)

    OQT, OKT, OV, OOT = 0, HEADS * DIM, 0, HEADS * DIM

    prev_exit = tile.TileContext._drain_and_barrier
    tile.TileContext._drain_and_barrier = _minimal_exit
    try:
        _build_body(nc, tc_args=(x_d, xt_d, wkq_d, wvo_d, b_d, y_d, OQT, OKT, OV, OOT))
    finally:
        tile.TileContext._drain_and_barrier = prev_exit

    # NOTE: hoisting startup work before the init barrier was tried and lost
    # time — the runtime preamble (~6.5us) gates all engines anyway, and
    # pre-barrier work just delays the barrier release for everyone.
    _split_multi_waits(nc)
    return nc


def _build_body(nc, tc_args):
    x_d, xt_d, wkq_d, wvo_d, b_d, y_d, OQT, OKT, OV, OOT = tc_args
    with tile.TileContext(nc) as tc:
        with (
            tc.tile_pool(name="wpool", bufs=1) as wpool,
            tc.tile_pool(name="spool", bufs=2) as spool,
            tc.tile_pool(name="ypool", bufs=2) as ypool,
            tc.tile_pool(name="psum", bufs=1, space="PSUM") as psum,
        ):
            hoist = []

            # ---- PE warmup: ~3.5us of dummy matmuls during the input DMAs
            # releases the HAM clock-gate so G runs at 2.4 GHz from its
            # first instruction.
            warm = wpool.tile([P, P], BF16, tag="warm")
            hoist.append(nc.gpsimd.memset(warm[:], 0).ins.name)
            wps = psum.tile([P, P], F32, tag="g0", bufs=1)
            for _ in range(24):
                hoist.append(
                    nc.tensor.matmul(wps[:], warm[:], warm[:], start=True, stop=True).ins.name
                )

            # ---- loads (xT slabs first: G consumes them incrementally;
            # triggers split across the two HWDGE engines (SP + ACT) so the
            # trigger chains run in parallel) ----
            xt_sb = []
            for qi in range(NQ):
                t = wpool.tile([P, NT // NQ, DIM], BF16, tag=f"xt{qi}")
                eng = nc.sync if qi % 2 == 0 else nc.scalar
                hoist.append(eng.dma_start(t[:], xt_d[qi]).ins.name)
                xt_sb.append(t)
            wkq_sb = []
            for k in range(KT):
                t = wpool.tile([P, 2 * HEADS * DIM], BF16, tag=f"wkq{k}")
                eng = nc.sync if k == 0 else nc.scalar
                eng.dma_start(t[:], wkq_d[k])
                wkq_sb.append(t)
            wvo_sb = []
            for k in range(KT):
                t = wpool.tile([P, 2 * HEADS * DIM], BF16, tag=f"wvo{k}")
                eng = nc.sync if k == 0 else nc.scalar
                eng.dma_start(t[:], wvo_d[k])
                wvo_sb.append(t)
            x_sb = []
            for k in range(KT):
                t = wpool.tile([P, N], BF16, tag=f"x{k}")
                eng = nc.sync if k == 0 else nc.scalar
                eng.dma_start(t[:], x_d[ts(k, P), :])
                x_sb.append(t)
            b_sb = []
            for ot in range(KT):
                t = wpool.tile([P, 1], F32, tag=f"b{ot}")
                eng = nc.sync if ot == 0 else nc.scalar
                eng.dma_start(t[:], b_d[ts(ot, P), :])
                b_sb.append(t)
            nc._hoist_names = hoist

            # ---- G = x @ x.T (fp32 PSUM, 32 accumulation steps) ----------
            g_ps = []
            for ct in range(KT):
                gp = psum.tile([P, DIM], F32, tag=f"g{ct}", bufs=1)
                g_ps.append(gp)
            for i in range(NT):
                qi, a = divmod(i, NT // NQ)
                for ct in range(KT):
                    nc.tensor.matmul(
                        g_ps[ct][:],
                        xt_sb[qi][:, a, ts(ct, P)],
                        xt_sb[qi][:, a, :],
                        start=(i == 0),
                        stop=(i == NT - 1),
                    )
            g_sb = []
            for ct in range(KT):
                g = spool.tile([P, DIM], BF16, tag=f"gs{ct}", bufs=1, name=f"g{ct}")
                nc.any.tensor_copy(g[:], g_ps[ct][:])
                g_sb.append(g)

            # ---- per-head stages, software-pipelined across heads --------
            # stage A(h): A = G @ Wk_h.T          (PE + drain)
            # stage L(h): L = (Wq_h*scale) @ A    (PE -> PSUM) + softmax
            # stage M(h): M_hT = Ehat . WoT       (PE + drain)
            a_all, es_all, lp_all = {}, {}, {}
            m_sb = {}

            def stage_A(h):
                a_sb = []
                for ct in range(KT):
                    ap = psum.tile([P, DIM], F32, tag="a", bufs=2, name=f"ap{h}_{ct}")
                    for k in range(KT):
                        # A[c', d] = sum_c'' G[c'', c'] wkT[c'', d]  (G symmetric)
                        nc.tensor.matmul(
                            ap[:],
                            g_sb[k][:, ts(ct, P)],
                            wkq_sb[k][:, OKT + h * DIM : OKT + (h + 1) * DIM],
                            start=(k == 0),
                            stop=(k == KT - 1),
                        )
                    at = spool.tile([P, DIM], BF16, tag=f"a{ct}", name=f"at{h}_{ct}")
                    nc.any.tensor_copy(at[:], ap[:])
                    a_sb.append(at)
                a_all[h] = a_sb

            def stage_L(h):
                pl = []
                for ct in range(KT):
                    lp = psum.tile([P, DIM], F32, tag=f"l{ct}", bufs=1, name=f"lp{h}_{ct}")
                    for k in range(KT):
                        # L[c, d] = sum_c' wqT[c', c] A[c', d]
                        nc.tensor.matmul(
                            lp[:],
                            wkq_sb[k][:, OQT + h * DIM + ct * P : OQT + h * DIM + (ct + 1) * P],
                            a_all[h][k][:],
                            start=(k == 0),
                            stop=(k == KT - 1),
                        )
                    pl.append(lp)
                lp_all[h] = pl
                # softmax immediately (ACT/DVE; doesn't occupy the PE)
                es = []
                for ct in range(KT):
                    e = spool.tile([P, DIM], BF16, tag=f"e{ct}", name=f"e{h}_{ct}")
                    s = spool.tile([P, 1], F32, tag=f"s{ct}", name=f"s{h}_{ct}")
                    r = spool.tile([P, 1], F32, tag=f"r{ct}", name=f"r{h}_{ct}")
                    nc.scalar.activation(
                        e[:], pl[ct][:], mybir.ActivationFunctionType.Exp,
                        accum_out=s[:],
                    )
                    nc.vector.reciprocal(r[:], s[:])
                    nc.any.tensor_scalar_mul(e[:], e[:], r[:])
                    es.append(e)
                es_all[h] = es

            def stage_M(h):
                es = es_all[h]
                for dt2 in range(KT):
                    pm = psum.tile([P, DIM], F32, tag="m", bufs=2, name=f"pm{h}_{dt2}")
                    for ct in range(KT):
                        # M_hT[d, o] = sum_c Ehat[c, d] woT[c, o]
                        nc.tensor.matmul(
                            pm[:],
                            es[ct][:, ts(dt2, P)],
                            wvo_sb[ct][:, OOT + h * DIM : OOT + (h + 1) * DIM],
                            start=(ct == 0),
                            stop=(ct == KT - 1),
                        )
                    mt = spool.tile([P, DIM], BF16, tag=f"m{h}_{dt2}", bufs=1,
                                    name=f"mt{h}_{dt2}")
                    m_sb[(h, dt2)] = mt
                    nc.any.tensor_copy(mt[:], pm[:])

            # pipelined emission: PE order A0 A1 L0 A2 L1 M0 A3 L2 M1 L3 M2 M3
            stage_A(0)
            stage_A(1)
            stage_L(0)
            stage_A(2)
            stage_L(1)
            stage_M(0)
            stage_A(3)
            stage_L(2)
            stage_M(1)
            stage_L(3)
            stage_M(2)
            stage_M(3)

            # ---- WstarT[c_in, o] = sum_h sum_d wv[d, c_in] M_hT[d, o] ----
            wst_sb = []
            for ct in range(KT):
                wp = psum.tile([P, DIM], F32, tag=f"l{ct}", bufs=1, name=f"wp{ct}")
                first = True
                for h in range(HEADS):
                    for dt2 in range(KT):
                        nc.tensor.matmul(
                            wp[:],
                            wvo_sb[dt2][:, OV + h * DIM + ct * P : OV + h * DIM + (ct + 1) * P],
                            m_sb[(h, dt2)][:],
                            start=first,
                            stop=(h == HEADS - 1 and dt2 == KT - 1),
                        )
                        first = False
                wt = spool.tile([P, DIM], BF16, tag=f"wst{ct}", bufs=1, name=f"wt{ct}")
                nc.any.tensor_copy(wt[:], wp[:])
                wst_sb.append(wt)

            # ---- y = WstarT.T @ x + b ------------------------------------
            # drains alternate DVE/ACT; 4 output DMAs of [128, 2048] each
            y_sb = {}
            for ot in range(KT):
                y_sb[ot] = ypool.tile([P, N], F32, tag=f"y{ot}", bufs=1,
                                      name=f"ysb{ot}")
            # store groups (in chunks): tapered so the final transfer is small
            store_after = {2: (0, 3), 5: (3, 3), 6: (6, 1), 7: (7, 1)}
            ycnt = 0
            ytags = ["g0", "g1", "a", "a"]
            for j in range(NCH):
                for ot in range(KT):
                    py = psum.tile([P, 512], F32, tag=ytags[ycnt % 4],
                                   bufs=(1 if ycnt % 4 < 2 else 2),
                                   name=f"py{j}_{ot}")
                    for k in range(KT):
                        nc.tensor.matmul(
                            py[:],
                            wst_sb[k][:, ts(ot, P)],
                            x_sb[k][:, ts(j, 512)],
                            start=(k == 0),
                            stop=(k == KT - 1),
                        )
                    dst = y_sb[ot][:, ts(j, 512)]
                    if ycnt % 2 == 0:
                        nc.vector.tensor_scalar_add(dst, py[:], b_sb[ot][:])
                    else:
                        nc.scalar.add(dst, py[:], b_sb[ot][:])
                    ycnt += 1
                    if j in store_after:
                        j0, nj = store_after[j]
                        nc.sync.dma_start(
                            y_d[ts(ot, P), j0 * 512 : (j0 + nj) * 512],
                            y_sb[ot][:, j0 * 512 : (j0 + nj) * 512],
                        )


def prep_inputs(x, w_qkv, w_out, b_out):
    """Host-side packing: per-core input dicts (numpy only)."""
    x = np.asarray(x, dtype=np.float32)
    w_qkv = np.asarray(w_qkv, dtype=np.float32)
    w_out = np.asarray(w_out, dtype=np.float32)
    b_out = np.asarray(b_out, dtype=np.float32)

    scale = float(DIM) ** -0.5
    wq = w_qkv[0 * HEADS * DIM : 1 * HEADS * DIM].reshape(HEADS, DIM, DIM)
    wk = w_qkv[1 * HEADS * DIM : 2 * HEADS * DIM].reshape(HEADS, DIM, DIM)
    wv = w_qkv[2 * HEADS * DIM : 3 * HEADS * DIM].reshape(HEADS, DIM, DIM)

    # wqT[c', h*256 + c] = wq[h, c, c'] * scale
    wqT = (np.transpose(wq, (2, 0, 1)) * scale).reshape(DIM, HEADS * DIM)
    # wkT[c', h*256 + d] = wk[h, d, c']
    wkT = np.transpose(wk, (2, 0, 1)).reshape(DIM, HEADS * DIM)
    # wvn[d, h*256 + c_in] = wv[h, d, c_in]  (natural orientation, head-concat)
    wvn = np.transpose(wv, (1, 0, 2)).reshape(DIM, HEADS * DIM)
    # woT[c, h*256 + o] = w_out[o, c*HEADS + h]
    woT = np.ascontiguousarray(
        w_out.reshape(DIM, DIM, HEADS).transpose(1, 2, 0)
    ).reshape(DIM, HEADS * DIM)

    # wkq[k] = [wqT | wkT], wvo[k] = [wv | woT], rows k*128:(k+1)*128
    wkq = np.ascontiguousarray(
        np.concatenate([wqT, wkT], axis=1).astype(NPBF16).reshape(KT, P, 2 * HEADS * DIM)
    )
    wvo = np.ascontiguousarray(
        np.concatenate([wvn, woT], axis=1).astype(NPBF16).reshape(KT, P, 2 * HEADS * DIM)
    )
    b = b_out.reshape(DIM, 1).astype(np.float32)

    in_maps = []
    for bi in range(B):
        xb = np.ascontiguousarray(x[bi].reshape(DIM, N)).astype(NPBF16)
        # xt[qi, p, a, c] = x.T[qi*1024 + a*128 + p, c]
        xt = np.ascontiguousarray(
            xb.T.reshape(NQ, NT // NQ, P, DIM).transpose(0, 2, 1, 3)
        )
        in_maps.append({"x": xb, "xt": xt, "wkq": wkq, "wvo": wvo, "b": b})
    return in_maps


_NC_CACHE = {}


def get_program():
    if "nc" not in _NC_CACHE:
        _NC_CACHE["nc"] = build_program()
    return _NC_CACHE["nc"]


def kernel(x, w_qkv, w_out, b_out, **_unused):
    nc = get_program()
    in_maps = prep_inputs(x, w_qkv, w_out, b_out)
    res = run_bass_kernel_spmd(nc, in_maps, list(range(N_CORES)))
    y = np.stack([res.results[c]["y"] for c in range(N_CORES)], axis=0)
    return y.reshape(B, DIM, H, W).astype(np.float32)


# revision 33
# speedup vs baseline: 1.1274x; 1.0173x over previous
"""Multi-head channel-attention kernel for Trainium2 (8 NeuronCores, SPMD).

Reference computation (per batch b, x = [256, N] with N = 64*64 = 4096):
    qkv   = w_qkv @ x
    q,k,v = per-head [256, N] slices of qkv
    logit = (q*scale) @ k.T          # [256, 256] (contraction over N)
    wts   = softmax(logit, -1)
    out_h = wts @ v
    y     = w_out @ stack_h(out_h) + b_out

Distribution: pure data-parallel — batch 8 across 8 cores, one batch per
core, no collectives.

The kernel exploits that attention is over the *channel* axis (n >> c):

    logit_h = (Wq_h * scale) @ (x @ x.T) @ Wk_h.T
    y       = (sum_h W_h @ softmax_h @ Wv_h) @ x + b  =  Wstar @ x + b

so the only n-wide work is the Gram matrix G = x @ x.T (one pass over x)
and the final Wstar @ x (second pass). Everything else is [256,256]-sized.
Per-batch FLOPs drop from 12.9G (direct) to 1.6G.

Pipeline (all matmuls TensorE, bf16 operands, fp32 PSUM):
    G    = xT.T @ xT                  (xT shipped pre-transposed from host)
    A_h  = G @ Wk_h.T                 (uses G's symmetry: lhsT = G)
    L_h  = (Wq_h*scale) @ A_h         -> PSUM
    E_h  = exp(L_h) on ScalarE straight from PSUM, accum_out = row sums;
           row-normalize with VectorE reciprocal (softmax; logits are O(1)
           for this problem so no max-subtraction is needed)
    M_hT = E_h-contraction with WoT   (computed directly transposed:
           lhsT = Ehat, rhs = WoT — no on-chip transposes anywhere)
    WstarT = sum_h Wv_h-contraction with M_hT
    y    = WstarT.T @ x + b           (bias folded into the PSUM drains,
           which alternate VectorE/ScalarE so they keep up with the PE)

The four [256,256]-per-head stages are software-pipelined across heads
(emission order A0 A1 L0 A2 L1 M0 A3 L2 M1 L3 M2 M3 Wst) so the PE never
waits on the softmax chain of the head in flight.

Weights are repacked/pre-transposed on the host; x is shipped twice (native
and transposed, bf16) so the Gram matmul needs no on-chip transpose. Input
DMAs are packed into 9 descriptors, outputs into 4.
"""

import numpy as np
import ml_dtypes

import concourse.bass as bass
import concourse.mybir as mybir
import concourse.tile as tile
from concourse.bass import ts
from concourse.bass_utils import run_bass_kernel_spmd
from concourse.vector_clock import ScopedClock

B, DIM, H, W = 8, 256, 64, 64
HEADS = 4
N = H * W            # 4096
P = 128
KT = DIM // P        # 2 channel tiles
NT = N // P          # 32 n-tiles of 128
NQ = 8               # xT shipped in 8 slabs of 4 n-tiles
NCH = N // 512       # 8 n-chunks of 512
N_CORES = 8

F32 = mybir.dt.float32
BF16 = mybir.dt.bfloat16
NPBF16 = ml_dtypes.bfloat16


def _split_multi_waits(nc, max_waits=1):
    """The walrus build in this container rejects instructions carrying more
    than one sync-wait. Move excess waits onto same-engine carrier NOPs
    inserted immediately before the instruction (engines are in-order, so
    waiting earlier on the same stream is equivalent)."""
    n_split = 0
    for f in nc.m.functions:
        for bb in f.blocks:
            old = list(bb.instructions)
            new = []
            changed = False
            for inst in old:
                si = inst.sync_info
                waits = list(si.on_wait) if si and si.on_wait else []
                if len(waits) > max_waits:
                    changed = True
                    for w in waits[max_waits:]:
                        n_split += 1
                        new.append(
                            mybir.InstNoOp(
                                name=f"wsplit_{n_split}_{inst.name}",
                                engine=inst.engine,
                                ins=[],
                                outs=[],
                                sync_info=mybir.SyncInfo(on_wait=[w], on_update=[]),
                            )
                        )
                    inst.sync_info = mybir.SyncInfo(
                        on_wait=waits[:max_waits], on_update=si.on_update
                    )
                new.append(inst)
            if changed:
                bb.instructions = new
    return n_split


def _minimal_exit(self, tick_clock, wait_clock):
    """TileContext._drain_and_barrier replacement: one SP drain carrying the
    global-clock waits (split onto NOPs by _split_multi_waits afterwards).

    The stock exit adds two all-engine barriers and ~200 per-semaphore
    clears (~10 us). They are redundant here: the bass preamble range-clears
    the whole kernel semaphore range at startup, and bass's own postamble
    still drains every engine.
    """
    nc = self.nc
    drain = nc.sync.drain()
    wait_clock.add_sem_waits(drain.ins, ScopedClock({None: tick_clock.global_clock}))
    popped = nc._tile_sem_poison_stack.pop()
    assert popped is self._sem_poison


def _hoist_to_main(nc):
    """Move dependency-free startup work (input-DMA triggers, PE warmup
    matmuls and their memset) from the tile block into `main`, before the
    startup all-engine barrier. The engines boot staggered over ~3-7 us;
    hoisted work overlaps that skew, so the input DMAs are in flight and
    the PE clock-gate is released by the time the real kernel body starts.
    Semaphores are cleared by the runtime at load, so pre-barrier sem
    increments are observed correctly by post-barrier waiters."""
    names = set(getattr(nc, "_hoist_names", []))
    if not names:
        return
    f = nc.m.functions[0]
    main = f.blocks[0]
    moved = []
    for b in f.blocks[1:]:
        old = list(b.instructions)
        got = [i for i in old if i.name in names]
        if got:
            b.instructions = [i for i in old if i.name not in names]
            moved.extend(got)
    main_list = list(main.instructions)
    # insert after the const-AP memsets, before the startup barrier drains
    pos = next(
        (
            idx
            for idx, i in enumerate(main_list)
            if i.sync_info
            and any("barrier" in (w.ant_name or "") for w in (i.sync_info.on_wait or []))
        ),
        len(main_list),
    )
    main.instructions = main_list[:pos] + moved + main_list[pos:]


def build_program():
    """Build the single-core Bass program (run SPMD across 8 cores)."""
    nc = bass.Bass()

    x_d = nc.declare_dram_parameter("x", [DIM, N], BF16, isOutput=False)
    # xt: [NQ][128, 8, 256]; slab qi, element (p, a, c) = x.T[qi*1024 + a*128 + p, c]
    xt_d = nc.declare_dram_parameter("xt", [NQ, P, NT // NQ, DIM], BF16, isOutput=False)
    # wkq: [KT][128, 2048] = [wqT | wkT]; wvo: [KT][128, 2048] = [wv | woT]
    wkq_d = nc.declare_dram_parameter("wkq", [KT, P, 2 * HEADS * DIM], BF16, isOutput=False)
    wvo_d = nc.declare_dram_parameter("wvo", [KT, P, 2 * HEADS * DIM], BF16, isOutput=False)
    b_d = nc.declare_dram_parameter("b", [DIM, 1], F32, isOutput=False)
    y_d = nc.declare_dram_parameter("y", [DIM, N], F32, isOutput=True)

    OQT, OKT, OV, OOT = 0, HEADS * DIM, 0, HEADS * DIM

    prev_exit = tile.TileContext._drain_and_barrier
    tile.TileContext._drain_and_barrier = _minimal_exit
    try:
        _build_body(nc, tc_args=(x_d, xt_d, wkq_d, wvo_d, b_d, y_d, OQT, OKT, OV, OOT))
    finally:
        tile.TileContext._drain_and_barrier = prev_exit

    # NOTE: hoisting startup work before the init barrier was tried and lost
    # time — the runtime preamble (~6.5us) gates all engines anyway, and
    # pre-barrier work just delays the barrier release for everyone.
    _split_multi_waits(nc)
    return nc


def _build_body(nc, tc_args):
    x_d, xt_d, wkq_d, wvo_d, b_d, y_d, OQT, OKT, OV, OOT = tc_args
    with tile.TileContext(nc) as tc:
        with (
            tc.tile_pool(name="wpool", bufs=1) as wpool,
            tc.tile_pool(name="spool", bufs=2) as spool,
            tc.tile_pool(name="ypool", bufs=2) as ypool,
            tc.tile_pool(name="psum", bufs=1, space="PSUM") as psum,
        ):
            hoist = []

            # ---- PE warmup: ~3.5us of dummy matmuls during the input DMAs
            # releases the HAM clock-gate so G runs at 2.4 GHz from its
            # first instruction.
            warm = wpool.tile([P, P], BF16, tag="warm")
            hoist.append(nc.gpsimd.memset(warm[:], 0).ins.name)
            wps = psum.tile([P, P], F32, tag="g0", bufs=1)
            for _ in range(24):
                hoist.append(
                    nc.tensor.matmul(wps[:], warm[:], warm[:], start=True, stop=True).ins.name
                )

            # ---- loads (xT slabs first: G consumes them incrementally;
            # triggers split across the two HWDGE engines (SP + ACT) so the
            # trigger chains run in parallel) ----
            xt_sb = []
            for qi in range(NQ):
                t = wpool.tile([P, NT // NQ, DIM], BF16, tag=f"xt{qi}")
                eng = nc.sync if qi % 2 == 0 else nc.scalar
                hoist.append(eng.dma_start(t[:], xt_d[qi]).ins.name)
                xt_sb.append(t)
            wkq_sb = []
            for k in range(KT):
                t = wpool.tile([P, 2 * HEADS * DIM], BF16, tag=f"wkq{k}")
                eng = nc.sync if k == 0 else nc.scalar
                eng.dma_start(t[:], wkq_d[k])
                wkq_sb.append(t)
            wvo_sb = []
            for k in range(KT):
                t = wpool.tile([P, 2 * HEADS * DIM], BF16, tag=f"wvo{k}")
                eng = nc.sync if k == 0 else nc.scalar
                eng.dma_start(t[:], wvo_d[k])
                wvo_sb.append(t)
            x_sb = []
            for k in range(KT):
                t = wpool.tile([P, N], BF16, tag=f"x{k}")
                eng = nc.sync if k == 0 else nc.scalar
                eng.dma_start(t[:], x_d[ts(k, P), :])
                x_sb.append(t)
            b_sb = []
            for ot in range(KT):
                t = wpool.tile([P, 1], F32, tag=f"b{ot}")
                eng = nc.sync if ot == 0 else nc.scalar
                eng.dma_start(t[:], b_d[ts(ot, P), :])
                b_sb.append(t)
            nc._hoist_names = hoist

            # ---- G = x @ x.T (fp32 PSUM, 32 accumulation steps) ----------
            g_ps = []
            for ct in range(KT):
                gp = psum.tile([P, DIM], F32, tag=f"g{ct}", bufs=1)
                g_ps.append(gp)
            for i in range(NT):
                qi, a = divmod(i, NT // NQ)
                for ct in range(KT):
                    nc.tensor.matmul(
                        g_ps[ct][:],
                        xt_sb[qi][:, a, ts(ct, P)],
                        xt_sb[qi][:, a, :],
                        start=(i == 0),
                        stop=(i == NT - 1),
                    )
            g_sb = []
            for ct in range(KT):
                g = spool.tile([P, DIM], BF16, tag=f"gs{ct}", bufs=1, name=f"g{ct}")
                nc.any.tensor_copy(g[:], g_ps[ct][:])
                g_sb.append(g)

            # ---- per-head stages, software-pipelined across heads --------
            # stage A(h): A = G @ Wk_h.T          (PE + drain)
            # stage L(h): L = (Wq_h*scale) @ A    (PE -> PSUM) + softmax
            # stage M(h): M_hT = Ehat . WoT       (PE + drain)
            a_all, es_all, lp_all = {}, {}, {}
            m_sb = {}

            def stage_A(h):
                a_sb = []
                for ct in range(KT):
                    ap = psum.tile([P, DIM], F32, tag="a", bufs=2, name=f"ap{h}_{ct}")
                    for k in range(KT):
                        # A[c', d] = sum_c'' G[c'', c'] wkT[c'', d]  (G symmetric)
                        nc.tensor.matmul(
                            ap[:],
                            g_sb[k][:, ts(ct, P)],
                            wkq_sb[k][:, OKT + h * DIM : OKT + (h + 1) * DIM],
                            start=(k == 0),
                            stop=(k == KT - 1),
                        )
                    at = spool.tile([P, DIM], BF16, tag=f"a{ct}", name=f"at{h}_{ct}")
                    nc.any.tensor_copy(at[:], ap[:])
                    a_sb.append(at)
                a_all[h] = a_sb

            def stage_L(h):
                pl = []
                for ct in range(KT):
                    lp = psum.tile([P, DIM], F32, tag=f"l{ct}", bufs=1, name=f"lp{h}_{ct}")
                    for k in range(KT):
                        # L[c, d] = sum_c' wqT[c', c] A[c', d]
                        nc.tensor.matmul(
                            lp[:],
                            wkq_sb[k][:, OQT + h * DIM + ct * P : OQT + h * DIM + (ct + 1) * P],
                            a_all[h][k][:],
                            start=(k == 0),
                            stop=(k == KT - 1),
                        )
                    pl.append(lp)
                lp_all[h] = pl
                # softmax immediately (ACT/DVE; doesn't occupy the PE)
                es = []
                for ct in range(KT):
                    e = spool.tile([P, DIM], BF16, tag=f"e{ct}", name=f"e{h}_{ct}")
                    s = spool.tile([P, 1], F32, tag=f"s{ct}", name=f"s{h}_{ct}")
                    r = spool.tile([P, 1], F32, tag=f"r{ct}", name=f"r{h}_{ct}")
                    nc.scalar.activation(
                        e[:], pl[ct][:], mybir.ActivationFunctionType.Exp,
                        accum_out=s[:],
                    )
                    nc.vector.reciprocal(r[:], s[:])
                    nc.any.tensor_scalar_mul(e[:], e[:], r[:])
                    es.append(e)
                es_all[h] = es

            def stage_M(h):
                es = es_all[h]
                for dt2 in range(KT):
                    pm = psum.tile([P, DIM], F32, tag="m", bufs=2, name=f"pm{h}_{dt2}")
                    for ct in range(KT):
                        # M_hT[d, o] = sum_c Ehat[c, d] woT[c, o]
                        nc.tensor.matmul(
                            pm[:],
                            es[ct][:, ts(dt2, P)],
                            wvo_sb[ct][:, OOT + h * DIM : OOT + (h + 1) * DIM],
                            start=(ct == 0),
                            stop=(ct == KT - 1),
                        )
                    mt = spool.tile([P, DIM], BF16, tag=f"m{h}_{dt2}", bufs=1,
                                    name=f"mt{h}_{dt2}")
                    m_sb[(h, dt2)] = mt
                    nc.any.tensor_copy(mt[:], pm[:])

            # pipelined emission: PE order A0 A1 L0 A2 L1 M0 A3 L2 M1 L3 M2 M3
            stage_A(0)
            stage_A(1)
            stage_L(0)
            stage_A(2)
            stage_L(1)
            stage_M(0)
            stage_A(3)
            stage_L(2)
            stage_M(1)
            stage_L(3)
            stage_M(2)
            stage_M(3)

            # ---- WstarT[c_in, o] = sum_h sum_d wv[d, c_in] M_hT[d, o] ----
            wst_sb = []
            for ct in range(KT):
                wp = psum.tile([P, DIM], F32, tag=f"l{ct}", bufs=1, name=f"wp{ct}")
                first = True
                for h in range(HEADS):
                    for dt2 in range(KT):
                        nc.tensor.matmul(
                            wp[:],
                            wvo_sb[dt2][:, OV + h * DIM + ct * P : OV + h * DIM + (ct + 1) * P],
                            m_sb[(h, dt2)][:],
                            start=first,
                            stop=(h == HEADS - 1 and dt2 == KT - 1),
                        )
                        first = False
                wt = spool.tile([P, DIM], BF16, tag=f"wst{ct}", bufs=1, name=f"wt{ct}")
                nc.any.tensor_copy(wt[:], wp[:])
                wst_sb.append(wt)

            # ---- y = WstarT.T @ x + b ------------------------------------
            # drains alternate DVE/ACT; 4 output DMAs of [128, 2048] each
            y_sb = {}
            for ot in range(KT):
                y_sb[ot] = ypool.tile([P, N], F32, tag=f"y{ot}", bufs=1,
                                      name=f"ysb{ot}")
            # store groups (in chunks): tapered so the final transfer is small
            store_after = {2: (0, 3), 5: (3, 3), 6: (6, 1), 7: (7, 1)}
            ycnt = 0
            ytags = ["g0", "g1", "a", "a"]
            for j in range(NCH):
                for ot in range(KT):
                    py = psum.tile([P, 512], F32, tag=ytags[ycnt % 4],
                                   bufs=(1 if ycnt % 4 < 2 else 2),
                                   name=f"py{j}_{ot}")
                    for k in range(KT):
                        nc.tensor.matmul(
                            py[:],
                            wst_sb[k][:, ts(ot, P)],
                            x_sb[k][:, ts(j, 512)],
                            start=(k == 0),
                            stop=(k == KT - 1),
                        )
                    dst = y_sb[ot][:, ts(j, 512)]
                    if ycnt % 2 == 0:
                        nc.vector.tensor_scalar_add(dst, py[:], b_sb[ot][:])
                    else:
                        nc.scalar.add(dst, py[:], b_sb[ot][:])
                    ycnt += 1
                    if j in store_after:
                        j0, nj = store_after[j]
                        nc.sync.dma_start(
                            y_d[ts(ot, P), j0 * 512 : (j0 + nj) * 512],
                            y_sb[ot][:, j0 * 512 : (j0 + nj) * 512],
                        )


def prep_inputs(x, w_qkv, w_out, b_out):
    """Host-side packing: per-core input dicts (numpy only)."""
    x = np.asarray(x, dtype=np.float32)
    w_qkv = np.asarray(w_qkv, dtype=np.float32)
    w_out = np.asarray(w_out, dtype=np.float32)
    b_out = np.asarray(b_out, dtype=np.float32)

    scale = float(DIM) ** -0.5
    wq = w_qkv[0 * HEADS * DIM : 1 * HEADS * DIM].reshape(HEADS, DIM, DIM)
    wk = w_qkv[1 * HEADS * DIM : 2 * HEADS * DIM].reshape(HEADS, DIM, DIM)
    wv = w_qkv[2 * HEADS * DIM : 3 * HEADS * DIM].reshape(HEADS, DIM, DIM)

    # wqT[c', h*256 + c] = wq[h, c, c'] * scale
    wqT = (np.transpose(wq, (2, 0, 1)) * scale).reshape(DIM, HEADS * DIM)
    # wkT[c', h*256 + d] = wk[h, d, c']
    wkT = np.transpose(wk, (2, 0, 1)).reshape(DIM, HEADS * DIM)
    # wvn[d, h*256 + c_in] = wv[h, d, c_in]  (natural orientation, head-concat)
    wvn = np.transpose(wv, (1, 0, 2)).reshape(DIM, HEADS * DIM)
    # woT[c, h*256 + o] = w_out[o, c*HEADS + h]
    woT = np.ascontiguousarray(
        w_out.reshape(DIM, DIM, HEADS).transpose(1, 2, 0)
    ).reshape(DIM, HEADS * DIM)

    # wkq[k] = [wqT | wkT], wvo[k] = [wv | woT], rows k*128:(k+1)*128
    wkq = np.ascontiguousarray(
        np.concatenate([wqT, wkT], axis=1).astype(NPBF16).reshape(KT, P, 2 * HEADS * DIM)
    )
    wvo = np.ascontiguousarray(
        np.concatenate([wvn, woT], axis=1).astype(NPBF16).reshape(KT, P, 2 * HEADS * DIM)
    )
    b = b_out.reshape(DIM, 1).astype(np.float32)

    in_maps = []
    for bi in range(B):
        xb = np.ascontiguousarray(x[bi].reshape(DIM, N)).astype(NPBF16)
        # xt[qi, p, a, c] = x.T[qi*1024 + a*128 + p, c]
        xt = np.ascontiguousarray(
            xb.T.reshape(NQ, NT // NQ, P, DIM).transpose(0, 2, 1, 3)
        )
        in_maps.append({"x": xb, "xt": xt, "wkq": wkq, "wvo": wvo, "b": b})
    return in_maps


_NC_CACHE = {}


def get_program():
    if "nc" not in _NC_CACHE:
        _NC_CACHE["nc"] = build_program()
    return _NC_CACHE["nc"]


def kernel(x, w_qkv, w_out, b_out, **_unused):
    nc = get_program()
    in_maps = prep_inputs(x, w_qkv, w_out, b_out)
    res = run_bass_kernel_spmd(nc, in_maps, list(range(N_CORES)))
    y = np.stack([res.results[c]["y"] for c in range(N_CORES)], axis=0)
    return y.reshape(B, DIM, H, W).astype(np.float32)
